# revision 8
# baseline (speedup 1.0000x reference)
"""Trainium2 Bass kernel for a GPT-2-style transformer block (B=2, S=2048, D=1024, H=16).

Sharding (8 cores): core c -> batch b=c//4, group position p=c%4.
 - Attention: head-parallel. Each core computes Q/K/V projections for its 4 heads
   over the full batch sequence, full causal attention for those heads, and a
   partial output projection [S, D]. A ReduceScatter over the 4 cores of the
   batch sums the partials and hands each core its 512-token slice.
 - FFN: token-parallel. Each core does LN1 + fc + gelu + mproj + LN2 for its
   512 tokens and writes its [512, 1024] output slice; the host concatenates.
"""

import math
import sys

import numpy as np

try:
    import concourse.bass as bass
except ImportError:
    sys.path.insert(0, "/opt/trn_rl_repo")
    import concourse.bass as bass

import concourse.tile as tile
from concourse import bacc, mybir
from concourse.bass_utils import run_bass_kernel_spmd
from concourse.masks import make_identity

F32 = mybir.dt.float32
AF = mybir.ActivationFunctionType
OP = mybir.AluOpType

B, S, D, H, DH = 2, 2048, 1024, 16, 64
N_CORES = 8
CPB = 4              # cores per batch (replica group size)
HPC = H // CPB       # heads per core = 4
GELU_C = math.sqrt(2.0 / math.pi)


def build_nc(seq=S, debug=False):
    """Build the SPMD kernel. `seq` is the per-batch sequence length (tunable
    for small-scale simulation); all other dims fixed."""
    NDT = D // 128           # 8   d-tiles
    NT = seq // 128          # k/t tiles over full sequence
    OWN = seq // CPB         # own tokens
    NOT_ = OWN // 128        # own t-tiles
    QCH = 512 if seq >= 512 else seq
    NQC = seq // QCH         # q chunks
    KTPQ = QCH // 128        # k-tiles per q chunk (diagonal region size)
    NHT = 4 * D // 128       # 32  hidden tiles
    NCOL = D // 512          # 2   512-col chunks of D
    eps = 1e-6

    nc = bacc.Bacc("TRN2", num_devices=N_CORES)

    # ---- I/O ----
    xT = nc.dram_tensor("xT", [D, seq], F32, kind="ExternalInput")
    x_own = nc.dram_tensor("x_own", [OWN, D], F32, kind="ExternalInput")
    w_qk = nc.dram_tensor("w_qk", [D, 2 * HPC * DH], F32, kind="ExternalInput")
    w_v = nc.dram_tensor("w_v", [D, HPC * DH], F32, kind="ExternalInput")
    w_ap = nc.dram_tensor("w_ap", [HPC * DH, D], F32, kind="ExternalInput")
    w_fc = nc.dram_tensor("w_fc", [D, 4 * D], F32, kind="ExternalInput")
    w_mp = nc.dram_tensor("w_mp", [4 * D, D], F32, kind="ExternalInput")
    masks = nc.dram_tensor("masks", [128, KTPQ, QCH], F32, kind="ExternalInput")
    bqk = nc.dram_tensor("bqk", [128, 4], F32, kind="ExternalInput")
    bv = nc.dram_tensor("bv", [128, HPC * DH], F32, kind="ExternalInput")
    bap = nc.dram_tensor("bap", [128, D], F32, kind="ExternalInput")  # pre-scaled by 1/CPB
    bfc = nc.dram_tensor("bfc", [128, NHT], F32, kind="ExternalInput")
    bmp = nc.dram_tensor("bmp", [128, D], F32, kind="ExternalInput")
    g1 = nc.dram_tensor("g1", [128, D], F32, kind="ExternalInput")
    b1 = nc.dram_tensor("b1", [128, D], F32, kind="ExternalInput")
    g2 = nc.dram_tensor("g2", [128, D], F32, kind="ExternalInput")
    b2 = nc.dram_tensor("b2", [128, D], F32, kind="ExternalInput")
    out_own = nc.dram_tensor("out_own", [OWN, D], F32, kind="ExternalOutput")
    if debug:
        dq = nc.dram_tensor("dq", [128, seq], F32, kind="ExternalOutput")
        dk = nc.dram_tensor("dk", [128, seq], F32, kind="ExternalOutput")
        dvz = nc.dram_tensor("dvz", [128, NT * HPC * 65], F32, kind="ExternalOutput")
        daT = nc.dram_tensor("daT", [128, seq], F32, kind="ExternalOutput")
        dpa = nc.dram_tensor("dpa", [128, NT * D], F32, kind="ExternalOutput")
        daown = nc.dram_tensor("daown", [128, NOT_ * D], F32, kind="ExternalOutput")
        dn = nc.dram_tensor("dn", [128, NOT_ * D], F32, kind="ExternalOutput")
        dht = nc.dram_tensor("dht", [128, NHT * OWN], F32, kind="ExternalOutput")

    with tile.TileContext(nc) as tc:
        with (
            tc.tile_pool(name="persist", bufs=1) as persist,
            tc.tile_pool(name="big", bufs=1) as bigpool,
            tc.tile_pool(name="dram", bufs=1, space="DRAM") as drampool,
        ):
            rs_in = drampool.tile([seq, D], F32)
            rs_out = drampool.tile([OWN, D], F32)

            # ---- constants / biases ----
            bqk_sb = persist.tile([128, 4], F32)
            nc.sync.dma_start(out=bqk_sb, in_=bqk.ap())
            bv_sb = persist.tile([128, HPC * DH], F32)
            nc.sync.dma_start(out=bv_sb, in_=bv.ap())
            bap_sb = persist.tile([128, D], F32)
            nc.sync.dma_start(out=bap_sb, in_=bap.ap())
            bfc_sb = persist.tile([128, NHT], F32)
            nc.sync.dma_start(out=bfc_sb, in_=bfc.ap())
            bmp_sb = persist.tile([128, D], F32)
            nc.sync.dma_start(out=bmp_sb, in_=bmp.ap())
            g1_sb = persist.tile([128, D], F32)
            nc.sync.dma_start(out=g1_sb, in_=g1.ap())
            b1_sb = persist.tile([128, D], F32)
            nc.sync.dma_start(out=b1_sb, in_=b1.ap())
            g2_sb = persist.tile([128, D], F32)
            nc.sync.dma_start(out=g2_sb, in_=g2.ap())
            b2_sb = persist.tile([128, D], F32)
            nc.sync.dma_start(out=b2_sb, in_=b2.ap())
            ident = persist.tile([128, 128], F32)
            make_identity(nc, ident)
            ones_row = persist.tile([128, 64], F32)
            nc.vector.memset(ones_row, 1.0)

            # ---- big shared-slot buffers (xT reused later by pa, then hT) ----
            xT_sb = bigpool.tile([128, NDT, seq], F32, tag="big8")
            nc.sync.dma_start(
                out=xT_sb, in_=xT.ap().rearrange("(dt p) s -> p dt s", p=128)
            )

            attnp = tc.alloc_tile_pool(name="attn", bufs=1)
            mask_sb = attnp.tile([128, KTPQ, QCH], F32)
            nc.sync.dma_start(out=mask_sb, in_=masks.ap())
            qkT_sb = [attnp.tile([128, seq], F32, name=f"qkT{i}", tag=f"qk{i}") for i in range(4)]
            vs_sb = attnp.tile([128, NT, HPC, 65], F32)
            nc.vector.memset(vs_sb[:, :, :, 64:65], 1.0)
            aT_sb = [attnp.tile([128, seq], F32, name=f"aT{i}", tag=f"at{i}") for i in range(2)]

            # ================= Stage A: QKV projections =================
            with (
                tc.tile_pool(name="wqkv", bufs=1) as wpool,
                tc.tile_pool(name="ps_a", bufs=2, space="PSUM") as ps_a,
            ):
                wqk_sb = wpool.tile([128, NDT, 2 * HPC * DH], F32)
                nc.sync.dma_start(
                    out=wqk_sb, in_=w_qk.ap().rearrange("(dt p) j -> p dt j", p=128)
                )
                wv_sb = wpool.tile([128, NDT, HPC * DH], F32)
                nc.sync.dma_start(
                    out=wv_sb, in_=w_v.ap().rearrange("(dt p) j -> p dt j", p=128)
                )
                # Q^T, K^T: [4 j-tiles of 128, seq]
                for jt in range(4):
                    for tch in range(seq // 512):
                        ps = ps_a.tile([128, 512], F32, tag="qk_ps")
                        for dt in range(NDT):
                            nc.tensor.matmul(
                                ps,
                                lhsT=wqk_sb[:, dt, jt * 128 : (jt + 1) * 128],
                                rhs=xT_sb[:, dt, tch * 512 : (tch + 1) * 512],
                                start=(dt == 0),
                                stop=(dt == NDT - 1),
                            )
                        nc.vector.tensor_scalar_add(
                            qkT_sb[jt][:, tch * 512 : (tch + 1) * 512],
                            ps,
                            bqk_sb[:, jt : jt + 1],
                        )
                # V natural + ones column: vs_sb[:, tt, h, 0:64]
                for tt in range(NT):
                    ps = ps_a.tile([128, HPC * DH], F32, tag="v_ps")
                    for dt in range(NDT):
                        nc.tensor.matmul(
                            ps,
                            lhsT=xT_sb[:, dt, tt * 128 : (tt + 1) * 128],
                            rhs=wv_sb[:, dt, :],
                            start=(dt == 0),
                            stop=(dt == NDT - 1),
                        )
                    for h in range(HPC):
                        nc.vector.tensor_tensor(
                            out=vs_sb[:, tt, h, 0:64],
                            in0=ps[:, h * 64 : (h + 1) * 64],
                            in1=bv_sb[:, h * 64 : (h + 1) * 64],
                            op=OP.add,
                        )

            # ================= Stage B: causal attention =================
            with (
                tc.tile_pool(name="pt", bufs=3) as ptpool,
                tc.tile_pool(name="sm", bufs=2) as smpool,
                tc.tile_pool(name="ps_st", bufs=3, space="PSUM") as ps_st,
                tc.tile_pool(name="ps_av", bufs=2, space="PSUM") as ps_av,
            ):
                for h in range(HPC):
                    qh = qkT_sb[h // 2][(h % 2) * 64 : (h % 2) * 64 + 64, :]
                    kh = qkT_sb[2 + h // 2][(h % 2) * 64 : (h % 2) * 64 + 64, :]
                    for qc in range(NQC):
                        apT = ps_av.tile([65, QCH], F32, tag="av")
                        nkt = KTPQ * (qc + 1)
                        for kt in range(nkt):
                            st = ps_st.tile([128, QCH], F32, tag="st")
                            nc.tensor.matmul(
                                st,
                                lhsT=kh[:, kt * 128 : (kt + 1) * 128],
                                rhs=qh[:, qc * QCH : (qc + 1) * QCH],
                                start=True,
                                stop=True,
                            )
                            pT = ptpool.tile([128, QCH], F32, tag="pt")
                            nc.scalar.activation(
                                out=pT, in_=st, func=AF.Exp, scale=1.0 / math.sqrt(DH)
                            )
                            od = kt - KTPQ * qc
                            if od >= 0:
                                nc.vector.tensor_mul(pT, pT, mask_sb[:, od, :])
                            nc.tensor.matmul(
                                apT,
                                lhsT=vs_sb[:, kt, h, :],
                                rhs=pT,
                                start=(kt == 0),
                                stop=(kt == nkt - 1),
                            )
                        nrm = smpool.tile([65, QCH], F32, tag="nrm")
                        nc.vector.reciprocal(nrm[64:65, :], apT[64:65, :])
                        # broadcast recip row (partition 64) to partitions 0..63 via PE
                        rb_ps = ps_av.tile([64, QCH], F32, tag="rb")
                        nc.tensor.matmul(
                            rb_ps, lhsT=ones_row[64:65, :], rhs=nrm[64:65, :],
                            start=True, stop=True,
                        )
                        rb_sb = smpool.tile([64, QCH], F32, tag="rb_sb")
                        nc.vector.tensor_copy(out=rb_sb, in_=rb_ps)
                        nc.vector.tensor_tensor(
                            out=nrm[0:64, :], in0=apT[0:64, :], in1=rb_sb, op=OP.mult
                        )
                        # DMA moves across partitions into the packed A^T tile
                        nc.sync.dma_start(
                            out=aT_sb[h // 2][
                                (h % 2) * 64 : (h % 2) * 64 + 64,
                                qc * QCH : (qc + 1) * QCH,
                            ],
                            in_=nrm[0:64, :],
                        )

            # ================= Stage C: partial aproj + ReduceScatter =================
            pa_sb = bigpool.tile([128, NT, D], F32, tag="big8")
            with (
                tc.tile_pool(name="wap", bufs=1) as wappool,
                tc.tile_pool(name="ps_c", bufs=2, space="PSUM") as ps_c,
            ):
                wap_sb = wappool.tile([128, 2, D], F32)
                nc.sync.dma_start(
                    out=wap_sb, in_=w_ap.ap().rearrange("(ft p) d -> p ft d", p=128)
                )
                for tt in range(NT):
                    for ncol in range(NCOL):
                        ps = ps_c.tile([128, 512], F32, tag="ap_ps")
                        for ft in range(2):
                            nc.tensor.matmul(
                                ps,
                                lhsT=aT_sb[ft][:, tt * 128 : (tt + 1) * 128],
                                rhs=wap_sb[:, ft, ncol * 512 : (ncol + 1) * 512],
                                start=(ft == 0),
                                stop=(ft == 1),
                            )
                        nc.vector.tensor_tensor(
                            out=pa_sb[:, tt, ncol * 512 : (ncol + 1) * 512],
                            in0=ps,
                            in1=bap_sb[:, ncol * 512 : (ncol + 1) * 512],
                            op=OP.add,
                        )
            if debug:
                nc.sync.dma_start(out=dq.ap(), in_=qkT_sb[0])
                nc.sync.dma_start(out=dk.ap(), in_=qkT_sb[2])
                nc.sync.dma_start(out=dvz.ap(), in_=vs_sb.rearrange("p a b c -> p (a b c)"))
                nc.sync.dma_start(out=daT.ap(), in_=aT_sb[0])
                nc.sync.dma_start(out=dpa.ap(), in_=pa_sb.rearrange("p a b -> p (a b)"))
            # funnel via one DMA, then ReduceScatter over the batch group
            nc.gpsimd.dma_start(
                out=rs_in[:].rearrange("(g p) d -> p g d", p=128), in_=pa_sb
            )
            attnp.release()
            nc.gpsimd.collective_compute(
                "ReduceScatter",
                OP.add,
                replica_groups=[[0, 1, 2, 3], [4, 5, 6, 7]],
                ins=[rs_in[:].opt()],
                outs=[rs_out[:].opt()],
            )

            # ================= Stage D: x + a, LN1 =================
            midp = tc.alloc_tile_pool(name="mid", bufs=1)
            n_sb = midp.tile([128, NOT_, D], F32)
            nT_sb = midp.tile([128, NDT, OWN], F32)
            with (
                tc.tile_pool(name="ln1", bufs=1) as lnpool,
                tc.tile_pool(name="ps_d", bufs=2, space="PSUM") as ps_d,
            ):
                a_own_sb = lnpool.tile([128, NOT_, D], F32, tag="aown")
                nc.gpsimd.dma_start(
                    out=a_own_sb, in_=rs_out[:].rearrange("(g p) d -> p g d", p=128)
                )
                x_own_sb = lnpool.tile([128, NOT_, D], F32, tag="xown")
                nc.sync.dma_start(
                    out=x_own_sb, in_=x_own.ap().rearrange("(g p) d -> p g d", p=128)
                )
                if debug:
                    nc.sync.dma_start(out=daown.ap(), in_=a_own_sb.rearrange("p a b -> p (a b)"))
                for tt in range(NOT_):
                    xa = lnpool.tile([128, D], F32, tag="xa")
                    nc.vector.tensor_tensor(
                        out=xa, in0=a_own_sb[:, tt, :], in1=x_own_sb[:, tt, :], op=OP.add
                    )
                    # LayerNorm (module-faithful: unbiased var, eps added to std)
                    scr = lnpool.tile([128, 16], F32, tag="scr")
                    stats = scr[:, 0:12].rearrange("p (s d) -> p s d", s=2)
                    xg = xa.rearrange("p (s d) -> p s d", s=2)
                    for sgi in range(2):
                        nc.vector.bn_stats(out=stats[:, sgi, :], in_=xg[:, sgi, :])
                    mv = scr[:, 12:14]
                    nc.vector.bn_aggr(out=mv, in_=stats)
                    std = scr[:, 14:15]
                    nc.scalar.activation(
                        out=std, in_=mv[:, 1:2], func=AF.Sqrt, scale=D / (D - 1.0)
                    )
                    nc.vector.tensor_scalar_add(std, std, eps)
                    rstd = scr[:, 15:16]
                    nc.vector.reciprocal(rstd, std)
                    nv = n_sb[:, tt, :]
                    nc.vector.tensor_scalar(
                        out=nv, in0=xa, scalar1=mv[:, 12:13] if False else scr[:, 12:13], scalar2=rstd,
                        op0=OP.subtract, op1=OP.mult,
                    )
                    nc.vector.tensor_mul(nv, nv, g1_sb)
                    nc.vector.tensor_add(nv, nv, b1_sb)
                    # transpose n tile -> nT
                    for dt in range(NDT):
                        tp = ps_d.tile([128, 128], F32, tag="tp")
                        nc.tensor.transpose(
                            tp, n_sb[:, tt, dt * 128 : (dt + 1) * 128], ident
                        )
                        nc.vector.tensor_copy(
                            out=nT_sb[:, dt, tt * 128 : (tt + 1) * 128], in_=tp
                        )

            if debug:
                nc.sync.dma_start(out=dn.ap(), in_=n_sb.rearrange("p a b -> p (a b)"))
            # ================= Stage E: fc + gelu =================
            hT_sb = bigpool.tile([128, NHT, OWN], F32, tag="big8")
            with (
                tc.tile_pool(name="wfc", bufs=2) as wfcpool,
                tc.tile_pool(name="gl", bufs=2) as glpool,
                tc.tile_pool(name="ps_e", bufs=2, space="PSUM") as ps_e,
            ):
                for ht in range(NHT):
                    wt = wfcpool.tile([128, NDT, 128], F32, tag="wfc")
                    nc.sync.dma_start(
                        out=wt,
                        in_=w_fc.ap()[:, ht * 128 : (ht + 1) * 128].rearrange(
                            "(dt p) c -> p dt c", p=128
                        ),
                    )
                    ps = ps_e.tile([128, OWN], F32, tag="fc_ps")
                    for dt in range(NDT):
                        nc.tensor.matmul(
                            ps,
                            lhsT=wt[:, dt, :],
                            rhs=nT_sb[:, dt, :],
                            start=(dt == 0),
                            stop=(dt == NDT - 1),
                        )
                    # gelu(tanh approx), bias folded in
                    xb = glpool.tile([128, OWN], F32, tag="xb")
                    nc.vector.tensor_scalar_add(xb, ps, bfc_sb[:, ht : ht + 1])
                    t2 = glpool.tile([128, OWN], F32, tag="t2")
                    nc.vector.tensor_mul(t2, xb, xb)
                    nc.vector.tensor_scalar(
                        out=t2, in0=t2, scalar1=0.044715, scalar2=1.0,
                        op0=OP.mult, op1=OP.add,
                    )
                    nc.vector.tensor_mul(t2, t2, xb)
                    nc.scalar.activation(out=t2, in_=t2, func=AF.Tanh, scale=GELU_C)
                    nc.vector.tensor_scalar(
                        out=t2, in0=t2, scalar1=1.0, scalar2=0.5,
                        op0=OP.add, op1=OP.mult,
                    )
                    nc.vector.tensor_mul(hT_sb[:, ht, :], t2, xb)

            if debug:
                nc.sync.dma_start(out=dht.ap(), in_=hT_sb.rearrange("p a b -> p (a b)"))
            # ================= Stage F: mproj + LN2 + out =================
            with (
                tc.tile_pool(name="wmp", bufs=2) as wmppool,
                tc.tile_pool(name="fin", bufs=1) as finpool,
                tc.tile_pool(name="ps_m", bufs=1, space="PSUM") as ps_m,
            ):
                m_ps = [
                    ps_m.tile([128, 512], F32, name=f"mps{i}", tag=f"m{i}")
                    for i in range(NOT_ * NCOL)
                ]
                for ht in range(NHT):
                    wt = wmppool.tile([128, D], F32, tag="wmp")
                    nc.sync.dma_start(out=wt, in_=w_mp.ap()[ht * 128 : (ht + 1) * 128, :])
                    for tt in range(NOT_):
                        for ncol in range(NCOL):
                            nc.tensor.matmul(
                                m_ps[tt * NCOL + ncol],
                                lhsT=hT_sb[:, ht, tt * 128 : (tt + 1) * 128],
                                rhs=wt[:, ncol * 512 : (ncol + 1) * 512],
                                start=(ht == 0),
                                stop=(ht == NHT - 1),
                            )
                for tt in range(NOT_):
                    msb = finpool.tile([128, D], F32, tag="msb")
                    for ncol in range(NCOL):
                        sl = slice(ncol * 512, (ncol + 1) * 512)
                        nc.vector.tensor_tensor(
                            out=msb[:, sl], in0=m_ps[tt * NCOL + ncol],
                            in1=n_sb[:, tt, sl], op=OP.add,
                        )
                    nc.vector.tensor_add(msb, msb, bmp_sb)
                    scr = finpool.tile([128, 16], F32, tag="scr2")
                    stats = scr[:, 0:12].rearrange("p (s d) -> p s d", s=2)
                    mg = msb.rearrange("p (s d) -> p s d", s=2)
                    for sgi in range(2):
                        nc.vector.bn_stats(out=stats[:, sgi, :], in_=mg[:, sgi, :])
                    mv = scr[:, 12:14]
                    nc.vector.bn_aggr(out=mv, in_=stats)
                    std = scr[:, 14:15]
                    nc.scalar.activation(
                        out=std, in_=mv[:, 1:2], func=AF.Sqrt, scale=D / (D - 1.0)
                    )
                    nc.vector.tensor_scalar_add(std, std, eps)
                    rstd = scr[:, 15:16]
                    nc.vector.reciprocal(rstd, std)
                    osb = finpool.tile([128, D], F32, tag="osb")
                    nc.vector.tensor_scalar(
                        out=osb, in0=msb, scalar1=scr[:, 12:13], scalar2=rstd,
                        op0=OP.subtract, op1=OP.mult,
                    )
                    nc.vector.tensor_mul(osb, osb, g2_sb)
                    nc.vector.tensor_add(osb, osb, b2_sb)
                    nc.sync.dma_start(
                        out=out_own.ap()[tt * 128 : (tt + 1) * 128, :], in_=osb
                    )
            midp.release()

    nc.compile()
    return nc


def make_in_maps(x, w_attn, b_attn, w_aproj, b_aproj, g1, b1, w_fc, b_fc,
                 w_mproj, b_mproj, g2, b2, seq=S):
    """Shard full inputs into the 8 per-core input maps."""
    OWN = seq // CPB
    QCH = 512 if seq >= 512 else seq
    KTPQ = QCH // 128
    x = np.ascontiguousarray(np.asarray(x, np.float32))
    w_attn = np.asarray(w_attn, np.float32)

    ones_b = lambda v: np.ascontiguousarray(
        np.broadcast_to(np.asarray(v, np.float32)[None, :], (128, v.shape[0]))
    )
    # diagonal masks: mask[k, o, q] = 1 if 128*o + k <= q
    kk = np.arange(128)[:, None, None]
    oo = np.arange(KTPQ)[None, :, None]
    qq = np.arange(QCH)[None, None, :]
    masks = ((128 * oo + kk) <= qq).astype(np.float32)

    in_maps = []
    for c in range(N_CORES):
        b, p = divmod(c, CPB)
        hs = slice(p * HPC * DH, (p + 1) * HPC * DH)
        xb = x[b]  # [seq, D]
        m = {
            "xT": np.ascontiguousarray(xb.T),
            "x_own": np.ascontiguousarray(xb[p * OWN : (p + 1) * OWN]),
            "w_qk": np.ascontiguousarray(
                np.concatenate([w_attn[:, hs], w_attn[:, D:][:, hs]], axis=1)
            ),
            "w_v": np.ascontiguousarray(w_attn[:, 2 * D :][:, hs]),
            "w_ap": np.ascontiguousarray(np.asarray(w_aproj, np.float32)[hs, :]),
            "w_fc": np.ascontiguousarray(np.asarray(w_fc, np.float32)),
            "w_mp": np.ascontiguousarray(np.asarray(w_mproj, np.float32)),
            "masks": np.ascontiguousarray(masks),
            "bqk": np.ascontiguousarray(
                np.concatenate(
                    [np.asarray(b_attn, np.float32)[hs],
                     np.asarray(b_attn, np.float32)[D:][hs]]
                ).reshape(4, 128).T
            ),
            "bv": ones_b(np.asarray(b_attn, np.float32)[2 * D :][hs]),
            "bap": ones_b(np.asarray(b_aproj, np.float32) / CPB),
            "bfc": np.ascontiguousarray(
                np.asarray(b_fc, np.float32).reshape(-1, 128).T
            ),
            "bmp": ones_b(np.asarray(b_mproj, np.float32)),
            "g1": ones_b(np.asarray(g1, np.float32)),
            "b1": ones_b(np.asarray(b1, np.float32)),
            "g2": ones_b(np.asarray(g2, np.float32)),
            "b2": ones_b(np.asarray(b2, np.float32)),
        }
        in_maps.append(m)
    return in_maps


def gather_out(results, seq=S):
    OWN = seq // CPB
    out = np.empty((B, seq, D), np.float32)
    for c in range(N_CORES):
        b, p = divmod(c, CPB)
        out[b, p * OWN : (p + 1) * OWN] = results[c]["out_own"]
    return out


_NC_CACHE = {}


def kernel(**inputs) -> np.ndarray:
    if "nc" not in _NC_CACHE:
        _NC_CACHE["nc"] = build_nc(S)
    nc = _NC_CACHE["nc"]
    in_maps = make_in_maps(**inputs)
    res = run_bass_kernel_spmd(nc, in_maps, core_ids=list(range(N_CORES)))
    return gather_out(res.results)


# revision 10
# speedup vs baseline: 140.2829x; 140.2829x over previous
"""Trainium2 Bass kernel for a GPT-2-style transformer block (B=2, S=2048, D=1024, H=16).

Sharding (8 cores): core c -> batch b=c//4, group position p=c%4.
 - Attention: head-parallel. Each core computes Q/K/V projections for its 4 heads
   over the full batch sequence, full causal attention for those heads, and a
   partial output projection [S, D]. A ReduceScatter over the 4 cores of the
   batch sums the partials and hands each core its 512-token slice.
 - FFN: token-parallel. Each core does LN1 + fc + gelu + mproj + LN2 for its
   512 tokens and writes its [512, 1024] output slice; the host concatenates.
"""

import math
import sys

import numpy as np

try:
    import concourse.bass as bass
except ImportError:
    sys.path.insert(0, "/opt/trn_rl_repo")
    import concourse.bass as bass

import concourse.tile as tile
from concourse import bacc, mybir
from concourse.bass_utils import run_bass_kernel_spmd
from concourse.masks import make_identity

F32 = mybir.dt.float32
AF = mybir.ActivationFunctionType
OP = mybir.AluOpType

B, S, D, H, DH = 2, 2048, 1024, 16, 64
N_CORES = 8
CPB = 4              # cores per batch (replica group size)
HPC = H // CPB       # heads per core = 4
GELU_C = math.sqrt(2.0 / math.pi)


def build_nc(seq=S, debug=False):
    """Build the SPMD kernel. `seq` is the per-batch sequence length (tunable
    for small-scale simulation); all other dims fixed."""
    NDT = D // 128           # 8   d-tiles
    NT = seq // 128          # k/t tiles over full sequence
    OWN = seq // CPB         # own tokens
    NOT_ = OWN // 128        # own t-tiles
    QCH = 512 if seq >= 512 else seq
    NQC = seq // QCH         # q chunks
    KTPQ = QCH // 128        # k-tiles per q chunk (diagonal region size)
    NHT = 4 * D // 128       # 32  hidden tiles
    NCOL = D // 512          # 2   512-col chunks of D
    eps = 1e-6

    nc = bacc.Bacc("TRN2", num_devices=N_CORES)

    # ---- I/O ----
    xT = nc.dram_tensor("xT", [D, seq], F32, kind="ExternalInput")
    x_own = nc.dram_tensor("x_own", [OWN, D], F32, kind="ExternalInput")
    w_qk = nc.dram_tensor("w_qk", [D, 2 * HPC * DH], F32, kind="ExternalInput")
    w_v = nc.dram_tensor("w_v", [D, HPC * DH], F32, kind="ExternalInput")
    w_ap = nc.dram_tensor("w_ap", [HPC * DH, D], F32, kind="ExternalInput")
    w_fc = nc.dram_tensor("w_fc", [D, 4 * D], F32, kind="ExternalInput")
    w_mp = nc.dram_tensor("w_mp", [4 * D, D], F32, kind="ExternalInput")
    masks = nc.dram_tensor("masks", [128, KTPQ, QCH], F32, kind="ExternalInput")
    bqk = nc.dram_tensor("bqk", [128, 4], F32, kind="ExternalInput")
    bv = nc.dram_tensor("bv", [128, HPC * DH], F32, kind="ExternalInput")
    bap = nc.dram_tensor("bap", [128, D], F32, kind="ExternalInput")  # pre-scaled by 1/CPB
    bfc = nc.dram_tensor("bfc", [128, NHT], F32, kind="ExternalInput")
    bmp = nc.dram_tensor("bmp", [128, D], F32, kind="ExternalInput")
    g1 = nc.dram_tensor("g1", [128, D], F32, kind="ExternalInput")
    b1 = nc.dram_tensor("b1", [128, D], F32, kind="ExternalInput")
    g2 = nc.dram_tensor("g2", [128, D], F32, kind="ExternalInput")
    b2 = nc.dram_tensor("b2", [128, D], F32, kind="ExternalInput")
    out_own = nc.dram_tensor("out_own", [OWN, D], F32, kind="ExternalOutput")
    if debug:
        dq = nc.dram_tensor("dq", [128, seq], F32, kind="ExternalOutput")
        dk = nc.dram_tensor("dk", [128, seq], F32, kind="ExternalOutput")
        dvz = nc.dram_tensor("dvz", [128, NT * HPC * 65], F32, kind="ExternalOutput")
        daT = nc.dram_tensor("daT", [128, seq], F32, kind="ExternalOutput")
        dpa = nc.dram_tensor("dpa", [128, NT * D], F32, kind="ExternalOutput")
        daown = nc.dram_tensor("daown", [128, NOT_ * D], F32, kind="ExternalOutput")
        dn = nc.dram_tensor("dn", [128, NOT_ * D], F32, kind="ExternalOutput")
        dht = nc.dram_tensor("dht", [128, NHT * OWN], F32, kind="ExternalOutput")

    with tile.TileContext(nc) as tc:
        with (
            tc.tile_pool(name="persist", bufs=1) as persist,
            tc.tile_pool(name="big", bufs=1) as bigpool,
            tc.tile_pool(name="dram", bufs=1, space="DRAM") as drampool,
        ):
            rs_in = drampool.tile([seq, D], F32)
            rs_out = drampool.tile([OWN, D], F32)

            # ---- constants / biases ----
            bqk_sb = persist.tile([128, 4], F32)
            nc.sync.dma_start(out=bqk_sb, in_=bqk.ap())
            bv_sb = persist.tile([128, HPC * DH], F32)
            nc.sync.dma_start(out=bv_sb, in_=bv.ap())
            bap_sb = persist.tile([128, D], F32)
            nc.sync.dma_start(out=bap_sb, in_=bap.ap())
            bfc_sb = persist.tile([128, NHT], F32)
            nc.sync.dma_start(out=bfc_sb, in_=bfc.ap())
            bmp_sb = persist.tile([128, D], F32)
            nc.sync.dma_start(out=bmp_sb, in_=bmp.ap())
            g1_sb = persist.tile([128, D], F32)
            nc.sync.dma_start(out=g1_sb, in_=g1.ap())
            b1_sb = persist.tile([128, D], F32)
            nc.sync.dma_start(out=b1_sb, in_=b1.ap())
            g2_sb = persist.tile([128, D], F32)
            nc.sync.dma_start(out=g2_sb, in_=g2.ap())
            b2_sb = persist.tile([128, D], F32)
            nc.sync.dma_start(out=b2_sb, in_=b2.ap())
            ident = persist.tile([128, 128], F32)
            make_identity(nc, ident)
            ones_row = persist.tile([128, 64], F32)
            nc.vector.memset(ones_row, 1.0)

            # ---- big shared-slot buffers (xT reused later by pa, then hT) ----
            xT_sb = bigpool.tile([128, NDT, seq], F32, tag="big8")
            nc.sync.dma_start(
                out=xT_sb, in_=xT.ap().rearrange("(dt p) s -> p dt s", p=128)
            )

            attnp = tc.alloc_tile_pool(name="attn", bufs=1)
            mask_sb = attnp.tile([128, KTPQ, QCH], F32)
            nc.sync.dma_start(out=mask_sb, in_=masks.ap())
            qkT_sb = [attnp.tile([128, seq], F32, name=f"qkT{i}", tag=f"qk{i}") for i in range(4)]
            vs_sb = attnp.tile([128, NT, HPC, 65], F32)
            nc.vector.memset(vs_sb[:, :, :, 64:65], 1.0)
            aT_sb = [attnp.tile([128, seq], F32, name=f"aT{i}", tag=f"at{i}") for i in range(2)]

            # ================= Stage A: QKV projections =================
            with (
                tc.tile_pool(name="wqkv", bufs=1) as wpool,
                tc.tile_pool(name="ps_a", bufs=2, space="PSUM") as ps_a,
            ):
                wqk_sb = wpool.tile([128, NDT, 2 * HPC * DH], F32)
                nc.sync.dma_start(
                    out=wqk_sb, in_=w_qk.ap().rearrange("(dt p) j -> p dt j", p=128)
                )
                wv_sb = wpool.tile([128, NDT, HPC * DH], F32)
                nc.sync.dma_start(
                    out=wv_sb, in_=w_v.ap().rearrange("(dt p) j -> p dt j", p=128)
                )
                # Q^T, K^T: [4 j-tiles of 128, seq]
                for jt in range(4):
                    for tch in range(seq // 512):
                        ps = ps_a.tile([128, 512], F32, tag="qk_ps")
                        for dt in range(NDT):
                            nc.tensor.matmul(
                                ps,
                                lhsT=wqk_sb[:, dt, jt * 128 : (jt + 1) * 128],
                                rhs=xT_sb[:, dt, tch * 512 : (tch + 1) * 512],
                                start=(dt == 0),
                                stop=(dt == NDT - 1),
                            )
                        nc.vector.tensor_scalar_add(
                            qkT_sb[jt][:, tch * 512 : (tch + 1) * 512],
                            ps,
                            bqk_sb[:, jt : jt + 1],
                        )
                # V natural + ones column: vs_sb[:, tt, h, 0:64]
                for tt in range(NT):
                    ps = ps_a.tile([128, HPC * DH], F32, tag="v_ps")
                    for dt in range(NDT):
                        nc.tensor.matmul(
                            ps,
                            lhsT=xT_sb[:, dt, tt * 128 : (tt + 1) * 128],
                            rhs=wv_sb[:, dt, :],
                            start=(dt == 0),
                            stop=(dt == NDT - 1),
                        )
                    for h in range(HPC):
                        nc.vector.tensor_tensor(
                            out=vs_sb[:, tt, h, 0:64],
                            in0=ps[:, h * 64 : (h + 1) * 64],
                            in1=bv_sb[:, h * 64 : (h + 1) * 64],
                            op=OP.add,
                        )

            # ================= Stage B: causal attention =================
            with (
                tc.tile_pool(name="pt", bufs=3) as ptpool,
                tc.tile_pool(name="sm", bufs=2) as smpool,
                tc.tile_pool(name="ps_st", bufs=3, space="PSUM") as ps_st,
                tc.tile_pool(name="ps_av", bufs=2, space="PSUM") as ps_av,
            ):
                for h in range(HPC):
                    qh = qkT_sb[h // 2][(h % 2) * 64 : (h % 2) * 64 + 64, :]
                    kh = qkT_sb[2 + h // 2][(h % 2) * 64 : (h % 2) * 64 + 64, :]
                    for qc in range(NQC):
                        apT = ps_av.tile([65, QCH], F32, tag="av")
                        nkt = KTPQ * (qc + 1)
                        for kt in range(nkt):
                            st = ps_st.tile([128, QCH], F32, tag="st")
                            nc.tensor.matmul(
                                st,
                                lhsT=kh[:, kt * 128 : (kt + 1) * 128],
                                rhs=qh[:, qc * QCH : (qc + 1) * QCH],
                                start=True,
                                stop=True,
                            )
                            pT = ptpool.tile([128, QCH], F32, tag="pt")
                            nc.scalar.activation(
                                out=pT, in_=st, func=AF.Exp, scale=1.0 / math.sqrt(DH)
                            )
                            od = kt - KTPQ * qc
                            if od >= 0:
                                nc.vector.tensor_mul(pT, pT, mask_sb[:, od, :])
                            nc.tensor.matmul(
                                apT,
                                lhsT=vs_sb[:, kt, h, :],
                                rhs=pT,
                                start=(kt == 0),
                                stop=(kt == nkt - 1),
                            )
                        nrm = smpool.tile([65, QCH], F32, tag="nrm")
                        nc.vector.reciprocal(nrm[64:65, :], apT[64:65, :])
                        # broadcast recip row (partition 64) to partitions 0..63 via PE
                        rb_ps = ps_av.tile([64, QCH], F32, tag="rb")
                        nc.tensor.matmul(
                            rb_ps, lhsT=ones_row[64:65, :], rhs=nrm[64:65, :],
                            start=True, stop=True,
                        )
                        rb_sb = smpool.tile([64, QCH], F32, tag="rb_sb")
                        nc.vector.tensor_copy(out=rb_sb, in_=rb_ps)
                        nc.vector.tensor_tensor(
                            out=nrm[0:64, :], in0=apT[0:64, :], in1=rb_sb, op=OP.mult
                        )
                        # DMA moves across partitions into the packed A^T tile
                        nc.sync.dma_start(
                            out=aT_sb[h // 2][
                                (h % 2) * 64 : (h % 2) * 64 + 64,
                                qc * QCH : (qc + 1) * QCH,
                            ],
                            in_=nrm[0:64, :],
                        )

            # ================= Stage C: partial aproj + ReduceScatter =================
            pa_sb = bigpool.tile([128, NT, D], F32, tag="big8")
            with (
                tc.tile_pool(name="wap", bufs=1) as wappool,
                tc.tile_pool(name="ps_c", bufs=2, space="PSUM") as ps_c,
            ):
                wap_sb = wappool.tile([128, 2, D], F32)
                nc.sync.dma_start(
                    out=wap_sb, in_=w_ap.ap().rearrange("(ft p) d -> p ft d", p=128)
                )
                for tt in range(NT):
                    for ncol in range(NCOL):
                        ps = ps_c.tile([128, 512], F32, tag="ap_ps")
                        for ft in range(2):
                            nc.tensor.matmul(
                                ps,
                                lhsT=aT_sb[ft][:, tt * 128 : (tt + 1) * 128],
                                rhs=wap_sb[:, ft, ncol * 512 : (ncol + 1) * 512],
                                start=(ft == 0),
                                stop=(ft == 1),
                            )
                        nc.vector.tensor_tensor(
                            out=pa_sb[:, tt, ncol * 512 : (ncol + 1) * 512],
                            in0=ps,
                            in1=bap_sb[:, ncol * 512 : (ncol + 1) * 512],
                            op=OP.add,
                        )
            if debug:
                nc.sync.dma_start(out=dq.ap(), in_=qkT_sb[0])
                nc.sync.dma_start(out=dk.ap(), in_=qkT_sb[2])
                nc.sync.dma_start(out=dvz.ap(), in_=vs_sb.rearrange("p a b c -> p (a b c)"))
                nc.sync.dma_start(out=daT.ap(), in_=aT_sb[0])
                nc.sync.dma_start(out=dpa.ap(), in_=pa_sb.rearrange("p a b -> p (a b)"))
            # funnel via one DMA, then ReduceScatter over the batch group
            nc.gpsimd.dma_start(
                out=rs_in[:].rearrange("(g p) d -> p g d", p=128), in_=pa_sb
            )
            attnp.release()
            nc.gpsimd.collective_compute(
                "ReduceScatter",
                OP.add,
                replica_groups=[[0, 1, 2, 3], [4, 5, 6, 7]],
                ins=[rs_in[:].opt()],
                outs=[rs_out[:].opt()],
            )

            # ================= Stage D: x + a, LN1 =================
            midp = tc.alloc_tile_pool(name="mid", bufs=1)
            n_sb = midp.tile([128, NOT_, D], F32)
            nT_sb = midp.tile([128, NDT, OWN], F32)
            with (
                tc.tile_pool(name="ln1", bufs=1) as lnpool,
                tc.tile_pool(name="ps_d", bufs=2, space="PSUM") as ps_d,
            ):
                a_own_sb = lnpool.tile([128, NOT_, D], F32, tag="aown")
                nc.gpsimd.dma_start(
                    out=a_own_sb, in_=rs_out[:].rearrange("(g p) d -> p g d", p=128)
                )
                x_own_sb = lnpool.tile([128, NOT_, D], F32, tag="xown")
                nc.sync.dma_start(
                    out=x_own_sb, in_=x_own.ap().rearrange("(g p) d -> p g d", p=128)
                )
                if debug:
                    nc.sync.dma_start(out=daown.ap(), in_=a_own_sb.rearrange("p a b -> p (a b)"))
                for tt in range(NOT_):
                    xa = lnpool.tile([128, D], F32, tag="xa")
                    nc.vector.tensor_tensor(
                        out=xa, in0=a_own_sb[:, tt, :], in1=x_own_sb[:, tt, :], op=OP.add
                    )
                    # LayerNorm (module-faithful: unbiased var, eps added to std)
                    scr = lnpool.tile([128, 16], F32, tag="scr")
                    stats = scr[:, 0:12].rearrange("p (s d) -> p s d", s=2)
                    xg = xa.rearrange("p (s d) -> p s d", s=2)
                    for sgi in range(2):
                        nc.vector.bn_stats(out=stats[:, sgi, :], in_=xg[:, sgi, :])
                    mv = scr[:, 12:14]
                    nc.vector.bn_aggr(out=mv, in_=stats)
                    std = scr[:, 14:15]
                    nc.scalar.activation(
                        out=std, in_=mv[:, 1:2], func=AF.Sqrt, scale=D / (D - 1.0)
                    )
                    nc.vector.tensor_scalar_add(std, std, eps)
                    rstd = scr[:, 15:16]
                    nc.vector.reciprocal(rstd, std)
                    nv = n_sb[:, tt, :]
                    nc.vector.tensor_scalar(
                        out=nv, in0=xa, scalar1=mv[:, 12:13] if False else scr[:, 12:13], scalar2=rstd,
                        op0=OP.subtract, op1=OP.mult,
                    )
                    nc.vector.tensor_mul(nv, nv, g1_sb)
                    nc.vector.tensor_add(nv, nv, b1_sb)
                    # transpose n tile -> nT
                    for dt in range(NDT):
                        tp = ps_d.tile([128, 128], F32, tag="tp")
                        nc.tensor.transpose(
                            tp, n_sb[:, tt, dt * 128 : (dt + 1) * 128], ident
                        )
                        nc.vector.tensor_copy(
                            out=nT_sb[:, dt, tt * 128 : (tt + 1) * 128], in_=tp
                        )

            if debug:
                nc.sync.dma_start(out=dn.ap(), in_=n_sb.rearrange("p a b -> p (a b)"))
            # ================= Stage E: fc + gelu =================
            hT_sb = bigpool.tile([128, NHT, OWN], F32, tag="big8")
            with (
                tc.tile_pool(name="wfc", bufs=2) as wfcpool,
                tc.tile_pool(name="gl", bufs=2) as glpool,
                tc.tile_pool(name="ps_e", bufs=2, space="PSUM") as ps_e,
            ):
                for ht in range(NHT):
                    wt = wfcpool.tile([128, NDT, 128], F32, tag="wfc")
                    nc.sync.dma_start(
                        out=wt,
                        in_=w_fc.ap()[:, ht * 128 : (ht + 1) * 128].rearrange(
                            "(dt p) c -> p dt c", p=128
                        ),
                    )
                    ps = ps_e.tile([128, OWN], F32, tag="fc_ps")
                    for dt in range(NDT):
                        nc.tensor.matmul(
                            ps,
                            lhsT=wt[:, dt, :],
                            rhs=nT_sb[:, dt, :],
                            start=(dt == 0),
                            stop=(dt == NDT - 1),
                        )
                    # gelu(tanh approx), bias folded in
                    xb = glpool.tile([128, OWN], F32, tag="xb")
                    nc.vector.tensor_scalar_add(xb, ps, bfc_sb[:, ht : ht + 1])
                    t2 = glpool.tile([128, OWN], F32, tag="t2")
                    nc.vector.tensor_mul(t2, xb, xb)
                    nc.vector.tensor_scalar(
                        out=t2, in0=t2, scalar1=0.044715, scalar2=1.0,
                        op0=OP.mult, op1=OP.add,
                    )
                    nc.vector.tensor_mul(t2, t2, xb)
                    nc.scalar.activation(out=t2, in_=t2, func=AF.Tanh, scale=GELU_C)
                    nc.vector.tensor_scalar(
                        out=t2, in0=t2, scalar1=1.0, scalar2=0.5,
                        op0=OP.add, op1=OP.mult,
                    )
                    nc.vector.tensor_mul(hT_sb[:, ht, :], t2, xb)

            if debug:
                nc.sync.dma_start(out=dht.ap(), in_=hT_sb.rearrange("p a b -> p (a b)"))
            # ================= Stage F: mproj + LN2 + out =================
            with (
                tc.tile_pool(name="wmp", bufs=2) as wmppool,
                tc.tile_pool(name="fin", bufs=1) as finpool,
                tc.tile_pool(name="ps_m", bufs=1, space="PSUM") as ps_m,
            ):
                m_ps = [
                    ps_m.tile([128, 512], F32, name=f"mps{i}", tag=f"m{i}")
                    for i in range(NOT_ * NCOL)
                ]
                for ht in range(NHT):
                    wt = wmppool.tile([128, D], F32, tag="wmp")
                    nc.sync.dma_start(out=wt, in_=w_mp.ap()[ht * 128 : (ht + 1) * 128, :])
                    for tt in range(NOT_):
                        for ncol in range(NCOL):
                            nc.tensor.matmul(
                                m_ps[tt * NCOL + ncol],
                                lhsT=hT_sb[:, ht, tt * 128 : (tt + 1) * 128],
                                rhs=wt[:, ncol * 512 : (ncol + 1) * 512],
                                start=(ht == 0),
                                stop=(ht == NHT - 1),
                            )
                for tt in range(NOT_):
                    msb = finpool.tile([128, D], F32, tag="msb")
                    for ncol in range(NCOL):
                        sl = slice(ncol * 512, (ncol + 1) * 512)
                        nc.vector.tensor_tensor(
                            out=msb[:, sl], in0=m_ps[tt * NCOL + ncol],
                            in1=n_sb[:, tt, sl], op=OP.add,
                        )
                    nc.vector.tensor_add(msb, msb, bmp_sb)
                    scr = finpool.tile([128, 16], F32, tag="scr2")
                    stats = scr[:, 0:12].rearrange("p (s d) -> p s d", s=2)
                    mg = msb.rearrange("p (s d) -> p s d", s=2)
                    for sgi in range(2):
                        nc.vector.bn_stats(out=stats[:, sgi, :], in_=mg[:, sgi, :])
                    mv = scr[:, 12:14]
                    nc.vector.bn_aggr(out=mv, in_=stats)
                    std = scr[:, 14:15]
                    nc.scalar.activation(
                        out=std, in_=mv[:, 1:2], func=AF.Sqrt, scale=D / (D - 1.0)
                    )
                    nc.vector.tensor_scalar_add(std, std, eps)
                    rstd = scr[:, 15:16]
                    nc.vector.reciprocal(rstd, std)
                    osb = finpool.tile([128, D], F32, tag="osb")
                    nc.vector.tensor_scalar(
                        out=osb, in0=msb, scalar1=scr[:, 12:13], scalar2=rstd,
                        op0=OP.subtract, op1=OP.mult,
                    )
                    nc.vector.tensor_mul(osb, osb, g2_sb)
                    nc.vector.tensor_add(osb, osb, b2_sb)
                    nc.sync.dma_start(
                        out=out_own.ap()[tt * 128 : (tt + 1) * 128, :], in_=osb
                    )
            midp.release()

    nc.compile()
    return nc


def make_in_maps(x, w_attn, b_attn, w_aproj, b_aproj, g1, b1, w_fc, b_fc,
                 w_mproj, b_mproj, g2, b2, seq=S):
    """Shard full inputs into the 8 per-core input maps."""
    OWN = seq // CPB
    QCH = 512 if seq >= 512 else seq
    KTPQ = QCH // 128
    x = np.ascontiguousarray(np.asarray(x, np.float32))
    w_attn = np.asarray(w_attn, np.float32)

    ones_b = lambda v: np.ascontiguousarray(
        np.broadcast_to(np.asarray(v, np.float32)[None, :], (128, v.shape[0]))
    )
    # diagonal masks: mask[k, o, q] = 1 if 128*o + k <= q
    kk = np.arange(128)[:, None, None]
    oo = np.arange(KTPQ)[None, :, None]
    qq = np.arange(QCH)[None, None, :]
    masks = ((128 * oo + kk) <= qq).astype(np.float32)

    in_maps = []
    for c in range(N_CORES):
        b, p = divmod(c, CPB)
        hs = slice(p * HPC * DH, (p + 1) * HPC * DH)
        xb = x[b]  # [seq, D]
        m = {
            "xT": np.ascontiguousarray(xb.T),
            "x_own": np.ascontiguousarray(xb[p * OWN : (p + 1) * OWN]),
            "w_qk": np.ascontiguousarray(
                np.concatenate([w_attn[:, hs], w_attn[:, D:][:, hs]], axis=1)
            ),
            "w_v": np.ascontiguousarray(w_attn[:, 2 * D :][:, hs]),
            "w_ap": np.ascontiguousarray(np.asarray(w_aproj, np.float32)[hs, :]),
            "w_fc": np.ascontiguousarray(np.asarray(w_fc, np.float32)),
            "w_mp": np.ascontiguousarray(np.asarray(w_mproj, np.float32)),
            "masks": np.ascontiguousarray(masks),
            "bqk": np.ascontiguousarray(
                np.concatenate(
                    [np.asarray(b_attn, np.float32)[hs],
                     np.asarray(b_attn, np.float32)[D:][hs]]
                ).reshape(4, 128).T
            ),
            "bv": ones_b(np.asarray(b_attn, np.float32)[2 * D :][hs]),
            "bap": ones_b(np.asarray(b_aproj, np.float32) / CPB),
            "bfc": np.ascontiguousarray(
                np.asarray(b_fc, np.float32).reshape(-1, 128).T
            ),
            "bmp": ones_b(np.asarray(b_mproj, np.float32)),
            "g1": ones_b(np.asarray(g1, np.float32)),
            "b1": ones_b(np.asarray(b1, np.float32)),
            "g2": ones_b(np.asarray(g2, np.float32)),
            "b2": ones_b(np.asarray(b2, np.float32)),
        }
        in_maps.append(m)
    return in_maps


def gather_out(results, seq=S):
    OWN = seq // CPB
    out = np.empty((B, seq, D), np.float32)
    for c in range(N_CORES):
        b, p = divmod(c, CPB)
        out[b, p * OWN : (p + 1) * OWN] = results[c]["out_own"]
    return out


_NC_CACHE = {}


def _get_runner():
    """Build the bass module once and return a cached jitted SPMD callable.

    Mirrors concourse.bass2jax.run_bass_via_pjrt but caches the traced/jitted
    function so repeat kernel() calls skip retracing and recompilation.
    """
    if "runner" in _NC_CACHE:
        return _NC_CACHE["runner"]
    import jax
    from jax.sharding import Mesh, PartitionSpec
    from jax.experimental.shard_map import shard_map
    from concourse import mybir as mb
    from concourse.bass2jax import (
        _bass_exec_p,
        install_neuronx_cc_hook,
        partition_id_tensor,
    )

    nc = build_nc(S)
    install_neuronx_cc_hook()

    partition_name = (
        nc.partition_id_tensor.name if nc.partition_id_tensor else None
    )
    in_names, out_names, out_avals, zero_outs = [], [], [], []
    for alloc in nc.m.functions[0].allocations:
        if not isinstance(alloc, mb.MemoryLocationSet):
            continue
        name = alloc.memorylocations[0].name
        if alloc.kind == "ExternalInput":
            if name != partition_name:
                in_names.append(name)
        elif alloc.kind == "ExternalOutput":
            shape = tuple(alloc.tensor_shape)
            dtype = mb.dt.np(alloc.dtype)
            out_names.append(name)
            out_avals.append(jax.core.ShapedArray(shape, dtype))
            zero_outs.append(np.zeros(shape, dtype))
    n_params = len(in_names)
    n_outs = len(out_avals)
    all_in_names = list(in_names) + list(out_names)
    if partition_name is not None:
        all_in_names.append(partition_name)
    donate = tuple(range(n_params, n_params + n_outs))

    def _body(*args):
        operands = list(args)
        if partition_name is not None:
            operands.append(partition_id_tensor())
        outs = _bass_exec_p.bind(
            *operands,
            out_avals=tuple(out_avals),
            in_names=tuple(all_in_names),
            out_names=tuple(out_names),
            lowering_input_output_aliases=(),
            sim_require_finite=True,
            sim_require_nnan=True,
            nc=nc,
        )
        return tuple(outs)

    devices = jax.devices()[:N_CORES]
    mesh = Mesh(np.asarray(devices), ("core",))
    in_specs = (PartitionSpec("core"),) * (n_params + n_outs)
    out_specs = (PartitionSpec("core"),) * n_outs
    sharded = jax.jit(
        shard_map(
            _body, mesh=mesh, in_specs=in_specs, out_specs=out_specs,
            check_rep=False,
        ),
        donate_argnums=donate,
        keep_unused=True,
    )
    runner = {
        "fn": sharded,
        "mesh": mesh,
        "in_names": in_names,
        "out_names": out_names,
        "out_avals": out_avals,
        "zero_shapes": [
            (N_CORES * z.shape[0], *z.shape[1:]) for z in zero_outs
        ],
        "zero_dtypes": [z.dtype for z in zero_outs],
    }
    _NC_CACHE["runner"] = runner
    return runner


def _concat_inputs(in_maps, in_names):
    return [
        np.concatenate([in_maps[c][name] for c in range(N_CORES)], axis=0)
        for name in in_names
    ]


def run_concat(concat_in):
    """Execute the kernel on pre-concatenated inputs; returns per-core results."""
    r = _get_runner()
    zeros = [
        np.zeros(sh, dt) for sh, dt in zip(r["zero_shapes"], r["zero_dtypes"])
    ]
    out_arrs = r["fn"](*concat_in, *zeros)
    results = []
    for c in range(N_CORES):
        results.append(
            {
                name: np.asarray(out_arrs[i]).reshape(
                    N_CORES, *r["out_avals"][i].shape
                )[c]
                for i, name in enumerate(r["out_names"])
            }
        )
    return results


def prepare(inputs):
    r = _get_runner()
    in_maps = make_in_maps(**inputs)
    return _concat_inputs(in_maps, r["in_names"])


def kernel(**inputs) -> np.ndarray:
    concat_in = prepare(inputs)
    return gather_out(run_concat(concat_in))
